# revision 13
# baseline (speedup 1.0000x reference)
"""Trainium2 Bass kernel for per-batch channel attention (CxAM-style).

Reference (per batch element b):
    q = (Wq @ x_b + bq)        # [64, T]
    k = (Wk @ x_b + bk)        # [64, T]
    v = (Wv @ x_b + bv)        # [512, T]
    R = q.T @ k                # [T, T]
    A = softmax(R, axis=-1)
    out_b = v @ A.T            # [512, T]

Sharding: pure data-parallel — batch B=8, one batch element per NeuronCore.

Per-core algorithm:
    x is DMA'd in four t-chunks (all channels per chunk) so the Q/K/V
    projections and t-block 0's score/AV pipeline chase the DMA.  All
    weight transposes go through the DMA xbar (dma_start_transpose, bf16)
    so the PE never transposes anything.  The K rows of the packed Q/K
    projection are written twice by ACT (once into qk rows 64:128, once
    onto partitions 0:64 as kq0) so score matmuls have both operands on
    matching partitions without any SBUF-duplication DMA.
    Per t-block of 512, per s-chunk pair:
      ST_j [s=128, t=512] = K_chunk.T @ Q_block      (scores, transposed)
      E_j = exp(ST_j)   one batched ACT op per pair  (bf16)
      U_ck [c=128, t] += VT_chunk_ck.T @ E_j         (unnormalized out)
    The U accumulation is split into halves A=(ck0,ck1)/B=(ck2,ck3); B of
    block tb runs during block tb+1, so only 2+2 PSUM banks are needed
    (4 remain for double-buffered score pairs).  When a half finishes it
    is immediately copied PSUM->SBUF, freeing the banks; normalization
    happens off the critical path: a DVE pairwise tree sums the 16 E_j
    tiles, a GPSIMD partition_all_reduce folds the 128 partitions
    (broadcasting the result), a DVE fast reciprocal gives rb, and
    out = U_sbuf * rb.  The last block instead uses staged partial tree
    sums plus a PE ones-matmul fold/broadcast (PSUM is free by then) so
    the tail chain after the final exp is short.  Output DMAs ride the
    sync ring (idle after the input phase) so they never delay the exps.
"""

import os

os.environ.setdefault("MYCRO_LOCAL_CACHE", "1")

import numpy as np

import concourse.bass as bass
import concourse.mybir as mybir
import concourse.tile as tile
from concourse import bacc
from concourse import bass_isa
from concourse.bass_utils import run_bass_kernel_spmd

F32 = mybir.dt.float32
BF16 = mybir.dt.bfloat16
AF = mybir.ActivationFunctionType

B = 8
C = 512
T = 2048
CQ = 64
NCORES = 8

TB = 512            # t-block (free dim of main matmuls)
NTB = T // TB       # 4
NSC = T // 128      # 16 s-chunks
NPAIR = NSC // 2    # 8 score pairs per t-block
NCH = C // 128      # 4 contraction chunks


def _build_program() -> bass.Bass:
    nc = bacc.Bacc("TRN2", target_bir_lowering=False, debug=False, num_devices=NCORES)

    x_d = nc.declare_dram_parameter("x", [C, T], F32, isOutput=False)
    wq_d = nc.declare_dram_parameter("Wq", [CQ, C], F32, isOutput=False)
    bq_d = nc.declare_dram_parameter("bq", [CQ, 1], F32, isOutput=False)
    wk_d = nc.declare_dram_parameter("Wk", [CQ, C], F32, isOutput=False)
    bk_d = nc.declare_dram_parameter("bk", [CQ, 1], F32, isOutput=False)
    wv_d = nc.declare_dram_parameter("Wv", [C, C], F32, isOutput=False)
    bv_d = nc.declare_dram_parameter("bv", [1, C], F32, isOutput=False)
    out_d = nc.declare_dram_parameter("out", [C, T], F32, isOutput=True)

    with tile.TileContext(nc) as tc:
        with (
            tc.tile_pool(name="const", bufs=1) as const,
            tc.tile_pool(name="weights", bufs=1) as wpool,
        ):
            ones_row = const.tile([1, 128], F32)
            nc.gpsimd.memset(ones_row[:], 1.0)
            ones_row_bf = const.tile([1, 128], BF16)
            nc.gpsimd.memset(ones_row_bf[:], 1.0)
            ones_col_bf = const.tile([128, 1], BF16)
            nc.gpsimd.memset(ones_col_bf[:], 1.0)

            # ---- input DMAs.  Sync ring: wq, wk, x0, wv, x1, x2, x3.
            # Bias loads and all transposes go on the ACT ring.
            wq_s = wpool.tile([CQ, C], F32)
            nc.sync.dma_start(out=wq_s[:], in_=wq_d[:])
            wk_s = wpool.tile([CQ, C], F32)
            nc.sync.dma_start(out=wk_s[:], in_=wk_d[:])
            wv_s = wpool.tile([128, NCH, C], F32)
            bqk = wpool.tile([128, 1], F32)    # [bq; bk]
            nc.scalar.dma_start(out=bqk[0:CQ, :], in_=bq_d[:])
            nc.scalar.dma_start(out=bqk[CQ:128, :], in_=bk_d[:])
            bv_row = wpool.tile([1, C], F32)
            nc.scalar.dma_start(out=bv_row[:], in_=bv_d[:])

            x_s = wpool.tile([128, NCH, T], F32)
            x_bf = wpool.tile([128, NCH, T], BF16)
            x_r = x_d[:].rearrange("(po pi) t -> pi po t", pi=128)

            # ---- weights to bf16, transposed through the DMA xbar
            wq_bf = wpool.tile([CQ, C], BF16)
            nc.vector.tensor_copy(wq_bf[:], wq_s[:])
            wk_bf = wpool.tile([CQ, C], BF16)
            nc.vector.tensor_copy(wk_bf[:], wk_s[:])
            wv_bf = wpool.tile([128, NCH, C], BF16)

            wqkT = wpool.tile([128, NCH, 128], BF16)  # [c, chunk, 0:64 WqT | 64:128 WkT]
            wvT = wpool.tile([128, NCH, C], BF16)     # [c, chunk, cout]
            nc.scalar.dma_start_transpose(
                out=wqkT[:, :, 0:CQ], in_=wq_bf[:]
            )
            nc.scalar.dma_start_transpose(
                out=wqkT[:, :, CQ:128], in_=wk_bf[:]
            )

            qk = wpool.tile([128, T], BF16)   # rows 0:64 Q, 64:128 K
            kq0 = wpool.tile([CQ, T], BF16)   # K duplicated onto partitions 0:64
            vT = wpool.tile([128, NSC, C], BF16)
            bv_bcast = wpool.tile([128, C], F32)

            # ---- main pools: 4 score banks + 2+2 AV banks
            with (
                tc.tile_pool(name="et", bufs=2) as et_pool,
                tc.tile_pool(name="ps", bufs=1, space="PSUM") as ps,
                tc.tile_pool(name="small", bufs=2) as small,
                tc.tile_pool(name="ubuf", bufs=2) as ubuf,
                tc.tile_pool(name="outp", bufs=2) as outp,
            ):
                avA = {}
                avB = {}
                ets = {}
                rbs = {}
                stage = {}

                # bv broadcast [1, C] -> [128, C] (borrows an AV bank)
                bvb = ps.tile([128, C], F32, tag="ava0", name="bvb")
                nc.tensor.matmul(
                    bvb[:], ones_row[:], bv_row[:], start=True, stop=True
                )
                nc.vector.tensor_copy(bv_bcast[:], bvb[:])

                def start_block(tb):
                    avA[tb] = [
                        ps.tile([128, TB], F32, tag=f"ava{ck}", name=f"avA{ck}_{tb}")
                        for ck in range(2)
                    ]
                    ets[tb] = et_pool.tile(
                        [128, NSC, TB], BF16, tag="et", name=f"et_{tb}"
                    )

                def emit_scores(tb, jp):
                    tsl = slice(tb * TB, (tb + 1) * TB)
                    j0, j1 = 2 * jp, 2 * jp + 1
                    sc = ps.tile(
                        [128, 2, TB], F32, tag="sc", bufs=2, name=f"sc_{tb}_{jp}"
                    )
                    nc.tensor.matmul(
                        sc[:, 0, :],
                        kq0[:, j0 * 128:(j0 + 1) * 128],
                        qk[0:CQ, tsl],
                        start=True,
                        stop=True,
                    )
                    nc.tensor.matmul(
                        sc[:, 1, :],
                        kq0[:, j1 * 128:(j1 + 1) * 128],
                        qk[0:CQ, tsl],
                        start=True,
                        stop=True,
                    )
                    nc.scalar.activation(
                        ets[tb][:, j0:j0 + 2, :], sc[:, :, :], AF.Exp
                    )

                def emit_consume_A(tb, jp):
                    for idx in (0, 1):
                        j = 2 * jp + idx
                        for ck in range(2):
                            nc.tensor.matmul(
                                avA[tb][ck][:],
                                vT[:, j, ck * 128:(ck + 1) * 128],
                                ets[tb][:, j, :],
                                start=(j == 0),
                                stop=(j == NSC - 1),
                            )

                def emit_consume_B(tb, jp):
                    if jp == 0:
                        avB[tb] = [
                            ps.tile([128, TB], F32, tag=f"avb{ck}",
                                    name=f"avB{ck}_{tb}")
                            for ck in range(2)
                        ]
                    for idx in (0, 1):
                        j = 2 * jp + idx
                        for ck in range(2):
                            nc.tensor.matmul(
                                avB[tb][ck][:],
                                vT[:, j, (2 + ck) * 128:(3 + ck) * 128],
                                ets[tb][:, j, :],
                                start=(j == 0),
                                stop=(j == NSC - 1),
                            )

                def emit_stage(tb, jp):
                    """Staged partial denominator sums for the last block."""
                    et = ets[tb]
                    if jp == 5:
                        s1 = small.tile([128, 4, TB], BF16, tag="s1", bufs=1)
                        nc.vector.tensor_add(s1[:], et[:, 0:4, :], et[:, 4:8, :])
                        s2 = small.tile([128, 2, TB], BF16, tag="s2", bufs=1)
                        nc.vector.tensor_add(s2[:], s1[:, 0:2, :], s1[:, 2:4, :])
                        stage["s2"] = s2
                    elif jp == 6:
                        s3 = small.tile([128, TB], BF16, tag="s3", bufs=1)
                        nc.vector.tensor_add(
                            s3[:], stage["s2"][:, 0, :], stage["s2"][:, 1, :]
                        )
                        s4 = small.tile([128, 2, TB], BF16, tag="s4", bufs=1)
                        nc.vector.tensor_add(s4[:], et[:, 8:10, :], et[:, 10:12, :])
                        s5 = small.tile([128, TB], BF16, tag="s5", bufs=1)
                        nc.vector.tensor_add(s5[:], s4[:, 0, :], s4[:, 1, :])
                        s011 = small.tile([128, TB], BF16, tag="s011", bufs=1)
                        nc.vector.tensor_add(s011[:], s3[:], s5[:])
                        stage["s011"] = s011

                def finish_A(tb):
                    """Drain A-half to SBUF (frees banks fast), then compute
                    the denominator reciprocal and normalize off-path."""
                    tsl = slice(tb * TB, (tb + 1) * TB)
                    uAt = [
                        ubuf.tile([128, TB], F32, tag=f"uA{i}", bufs=1,
                                  name=f"uA{i}_{tb}")
                        for i in range(2)
                    ]
                    for i in range(2):
                        nc.vector.tensor_copy(uAt[i][:], avA[tb][i][:])
                    et = ets[tb]
                    e8 = small.tile([128, 8, TB], BF16, tag="e8", bufs=1,
                                    name=f"e8_{tb}")
                    nc.vector.tensor_add(e8[:], et[:, 0:8, :], et[:, 8:16, :])
                    e4 = small.tile([128, 4, TB], BF16, tag="e4", bufs=1,
                                    name=f"e4_{tb}")
                    nc.vector.tensor_add(e4[:], e8[:, 0:4, :], e8[:, 4:8, :])
                    e2 = small.tile([128, 2, TB], BF16, tag="e2", bufs=1,
                                    name=f"e2_{tb}")
                    nc.vector.tensor_add(e2[:], e4[:, 0:2, :], e4[:, 2:4, :])
                    esum = small.tile([128, TB], F32, tag="esum", bufs=1,
                                      name=f"esum_{tb}")
                    nc.vector.tensor_add(esum[:], e2[:, 0, :], e2[:, 1, :])
                    dsum = small.tile([128, TB], F32, tag="dsum", bufs=1,
                                      name=f"dsum_{tb}")
                    nc.gpsimd.partition_all_reduce(
                        dsum[:], esum[:], channels=128,
                        reduce_op=bass_isa.ReduceOp.add,
                    )
                    rbs[tb] = small.tile([128, TB], F32, tag="rb", name=f"rb_{tb}")
                    nc.vector.reciprocal_approx_fast(rbs[tb][:], dsum[:])
                    for i in range(2):
                        ot = outp.tile(
                            [128, TB], F32, tag=f"ot{i}", name=f"ot{i}_{tb}"
                        )
                        nc.vector.tensor_mul(ot[:], uAt[i][:], rbs[tb][:])
                        nc.sync.dma_start(
                            out=out_d[i * 128:(i + 1) * 128, tsl], in_=ot[:]
                        )

                def finish_B(tb):
                    tsl = slice(tb * TB, (tb + 1) * TB)
                    uBt = [
                        ubuf.tile([128, TB], F32, tag=f"uB{i}", bufs=1,
                                  name=f"uB{i}_{tb}")
                        for i in range(2)
                    ]
                    for i in range(2):
                        nc.vector.tensor_copy(uBt[i][:], avB[tb][i][:])
                    for i in range(2):
                        ck = 2 + i
                        ot = outp.tile(
                            [128, TB], F32, tag=f"ot{ck}", name=f"ot{ck}_{tb}"
                        )
                        nc.vector.tensor_mul(ot[:], uBt[i][:], rbs[tb][:])
                        nc.sync.dma_start(
                            out=out_d[ck * 128:(ck + 1) * 128, tsl], in_=ot[:]
                        )

                def tail_last(tb):
                    """Tail for the final block: B-pairs interleave with the
                    PE denominator fold; muls read PSUM directly."""
                    tsl = slice(tb * TB, (tb + 1) * TB)
                    et = ets[tb]
                    # DVE: finish the staged denominator sum (waits last exp)
                    p1 = small.tile([128, 2, TB], BF16, tag="p1", bufs=1)
                    nc.vector.tensor_add(p1[:], et[:, 12:14, :], et[:, 14:16, :])
                    p2 = small.tile([128, TB], BF16, tag="p2", bufs=1)
                    nc.vector.tensor_add(p2[:], p1[:, 0, :], p1[:, 1, :])
                    esum = small.tile([128, TB], BF16, tag="esumL", bufs=1)
                    nc.vector.tensor_add(esum[:], stage["s011"][:], p2[:])
                    # PE: remaining B pairs around the denominator fold
                    emit_consume_B(tb, 4)
                    emit_consume_B(tb, 5)
                    dnrb = ps.tile([128, 2, TB], F32, tag="sc", bufs=2, name="dnrb")
                    nc.tensor.matmul(
                        dnrb[0:1, 0, :], ones_col_bf[:], esum[:],
                        start=True, stop=True,
                    )
                    dnrow = small.tile([1, TB], F32, tag="dnrow", bufs=1)
                    nc.scalar.copy(dnrow[:], dnrb[0:1, 0, :])
                    rcol = small.tile([1, TB], F32, tag="rcol", bufs=1)
                    nc.vector.reciprocal_approx_fast(rcol[:], dnrow[:])
                    rcol_bf = small.tile([1, TB], BF16, tag="rcolbf", bufs=1)
                    nc.vector.tensor_copy(rcol_bf[:], rcol[:])
                    emit_consume_B(tb, 6)
                    emit_consume_B(tb, 7)
                    nc.tensor.matmul(
                        dnrb[:, 1, :], ones_row_bf[:], rcol_bf[:],
                        start=True, stop=True,
                    )
                    rb = small.tile([128, TB], F32, tag="rbL", bufs=1)
                    nc.scalar.copy(rb[:], dnrb[:, 1, :])
                    for ck in range(4):
                        av = avA[tb][ck] if ck < 2 else avB[tb][ck - 2]
                        ot = outp.tile(
                            [128, TB], F32, tag=f"ot{ck}", name=f"otL{ck}_{tb}"
                        )
                        nc.vector.tensor_mul(ot[:], av[:], rb[:])
                        nc.sync.dma_start(
                            out=out_d[ck * 128:(ck + 1) * 128, tsl], in_=ot[:]
                        )

                # ---- preamble: x chunks with projections + t-block 0 chasing
                pending = None
                start_block(0)
                for m in range(NTB):
                    msl = slice(m * TB, (m + 1) * TB)
                    nc.sync.dma_start(out=x_s[:, :, msl], in_=x_r[:, :, msl])
                    if m == 0:
                        nc.sync.dma_start(
                            out=wv_s[:],
                            in_=wv_d[:].rearrange("(po pi) c -> pi po c", pi=128),
                        )
                    nc.vector.tensor_copy(
                        x_bf[:, :, m * TB:m * TB + 256], x_s[:, :, m * TB:m * TB + 256]
                    )
                    nc.scalar.activation(
                        x_bf[:, :, m * TB + 256:(m + 1) * TB],
                        x_s[:, :, m * TB + 256:(m + 1) * TB],
                        AF.Copy,
                    )

                    # packed Q/K projection; ACT writes qk and the kq0 dup
                    qkp = ps.tile([128, 2, TB], F32, tag="sc", bufs=2,
                                  name=f"qkp_{m}")
                    for ci in range(NCH):
                        nc.tensor.matmul(
                            qkp[:, 0, :], wqkT[:, ci, :], x_bf[:, ci, msl],
                            start=(ci == 0), stop=(ci == NCH - 1),
                        )
                    nc.scalar.add(qk[:, msl], qkp[:, 0, :], bqk[:, 0:1])
                    nc.scalar.add(kq0[:, msl], qkp[CQ:128, 0, :], bqk[CQ:128, 0:1])

                    if m == 0:
                        # wv arrives after x0: cast + xbar transpose
                        nc.vector.tensor_copy(wv_bf[:], wv_s[:])
                        for po in range(NCH):
                            nc.scalar.dma_start_transpose(
                                out=wvT[:, :, po * 128:(po + 1) * 128],
                                in_=wv_bf[:, po, :],
                            )

                    # t-block 0 scores chase the projections
                    for jp in (2 * m, 2 * m + 1):
                        emit_scores(0, jp)

                    # V^T projection for this chunk's four s-chunks
                    for j in range(4 * m, 4 * m + 4):
                        psv = ps.tile([128, C], F32, tag=f"avb{j % 2}",
                                      name=f"vp_{j}")
                        for ci in range(NCH):
                            nc.tensor.matmul(
                                psv[:],
                                x_bf[:, ci, j * 128:(j + 1) * 128],
                                wvT[:, ci, :],
                                start=(ci == 0),
                                stop=(ci == NCH - 1),
                            )
                        nc.vector.tensor_add(vT[:, j, :], psv[:], bv_bcast[:])

                    # consume t-block 0 pairs one step behind
                    for jp in (2 * m, 2 * m + 1):
                        if pending is not None:
                            emit_consume_A(*pending)
                        pending = (0, jp)

                # ---- main phases
                for tb in range(1, NTB):
                    start_block(tb)
                    for jp in range(NPAIR):
                        emit_scores(tb, jp)
                        if tb == NTB - 1:
                            emit_stage(tb, jp)
                        ptb, pjp = pending
                        emit_consume_A(ptb, pjp)
                        if pjp == NPAIR - 1:
                            finish_A(ptb)
                        pending = (tb, jp)
                        if tb < NTB - 1:
                            emit_consume_B(tb - 1, jp)
                            if jp == NPAIR - 1:
                                finish_B(tb - 1)
                        else:
                            # last phase: catch up — B(2) at 2 pairs/step,
                            # then start B(3)
                            if jp < 4:
                                emit_consume_B(tb - 1, 2 * jp)
                                emit_consume_B(tb - 1, 2 * jp + 1)
                                if jp == 3:
                                    finish_B(tb - 1)
                            else:
                                emit_consume_B(tb, jp - 4)
                # tail
                ptb, pjp = pending
                emit_consume_A(ptb, pjp)
                tail_last(NTB - 1)

    nc.compile()
    return nc


_PROGRAM = None


def _get_program() -> bass.Bass:
    global _PROGRAM
    if _PROGRAM is None:
        _PROGRAM = _build_program()
    return _PROGRAM


def kernel(**inputs: np.ndarray) -> np.ndarray:
    x = np.ascontiguousarray(np.asarray(inputs["x"], dtype=np.float32))
    wq = np.ascontiguousarray(np.asarray(inputs["Wq"], dtype=np.float32))
    bq = np.ascontiguousarray(np.asarray(inputs["bq"], dtype=np.float32)).reshape(CQ, 1)
    wk = np.ascontiguousarray(np.asarray(inputs["Wk"], dtype=np.float32))
    bk = np.ascontiguousarray(np.asarray(inputs["bk"], dtype=np.float32)).reshape(CQ, 1)
    wv = np.ascontiguousarray(np.asarray(inputs["Wv"], dtype=np.float32))
    bv = np.ascontiguousarray(np.asarray(inputs["bv"], dtype=np.float32)).reshape(1, C)

    nc = _get_program()
    in_maps = [
        {
            "x": np.ascontiguousarray(x[b]),
            "Wq": wq,
            "bq": bq,
            "Wk": wk,
            "bk": bk,
            "Wv": wv,
            "bv": bv,
        }
        for b in range(NCORES)
    ]
    res = run_bass_kernel_spmd(nc, in_maps, list(range(NCORES)))
    out = np.stack([res.results[b]["out"] for b in range(NCORES)], axis=0)
    return out.astype(np.float32)


if __name__ == "__main__":
    import reference

    inputs = {k: np.asarray(v) for k, v in reference.setup_inputs().items()}
    expected = np.asarray(reference.reference(**inputs))
    actual = kernel(**inputs)
    rel = np.linalg.norm(actual - expected) / np.linalg.norm(expected)
    print("Relative error:", rel)


# revision 14
# speedup vs baseline: 1.0960x; 1.0960x over previous
"""Trainium2 Bass kernel for per-batch channel attention (CxAM-style).

Reference (per batch element b):
    q = (Wq @ x_b + bq)        # [64, T]
    k = (Wk @ x_b + bk)        # [64, T]
    v = (Wv @ x_b + bv)        # [512, T]
    R = q.T @ k                # [T, T]
    A = softmax(R, axis=-1)
    out_b = v @ A.T            # [512, T]

Sharding: pure data-parallel — batch B=8, one batch element per NeuronCore.

Per-core algorithm:
    x is DMA'd in four t-chunks (all channels per chunk) so the Q/K/V
    projections and t-block 0's score/AV pipeline chase the DMA.  All
    weight transposes go through the DMA xbar (dma_start_transpose, bf16)
    so the PE never transposes anything.  The K rows of the packed Q/K
    projection are written twice by ACT (once into qk rows 64:128, once
    onto partitions 0:64 as kq0) so score matmuls have both operands on
    matching partitions without any SBUF-duplication DMA.
    Per t-block of 512, per s-chunk pair:
      ST_j [s=128, t=512] = K_chunk.T @ Q_block      (scores, transposed)
      E_j = exp(ST_j)   one batched ACT op per pair  (bf16)
      U_ck [c=128, t] += VT_chunk_ck.T @ E_j         (unnormalized out)
    The U accumulation is split into halves A=(ck0,ck1)/B=(ck2,ck3); B of
    block tb runs during block tb+1, so only 2+2 PSUM banks are needed
    (4 remain for double-buffered score pairs).  When a half finishes it
    is immediately copied PSUM->SBUF, freeing the banks; normalization
    happens off the critical path: a DVE pairwise tree sums the 16 E_j
    tiles, a GPSIMD partition_all_reduce folds the 128 partitions
    (broadcasting the result), a DVE fast reciprocal gives rb, and
    out = U_sbuf * rb.  The last block instead uses staged partial tree
    sums plus a PE ones-matmul fold/broadcast (PSUM is free by then) so
    the tail chain after the final exp is short.  Output DMAs ride the
    sync ring (idle after the input phase) so they never delay the exps.
"""

import os

os.environ.setdefault("MYCRO_LOCAL_CACHE", "1")

import numpy as np

import concourse.bass as bass
import concourse.mybir as mybir
import concourse.tile as tile
from concourse import bacc
from concourse import bass_isa
from concourse.bass_utils import run_bass_kernel_spmd
from concourse.masks import make_identity

F32 = mybir.dt.float32
BF16 = mybir.dt.bfloat16
AF = mybir.ActivationFunctionType

B = 8
C = 512
T = 2048
CQ = 64
NCORES = 8

TB = 512            # t-block (free dim of main matmuls)
NTB = T // TB       # 4
NSC = T // 128      # 16 s-chunks
NPAIR = NSC // 2    # 8 score pairs per t-block
NCH = C // 128      # 4 contraction chunks


def _build_program() -> bass.Bass:
    nc = bacc.Bacc("TRN2", target_bir_lowering=False, debug=False, num_devices=NCORES)

    x_d = nc.declare_dram_parameter("x", [C, T], F32, isOutput=False)
    wq_d = nc.declare_dram_parameter("Wq", [CQ, C], F32, isOutput=False)
    bq_d = nc.declare_dram_parameter("bq", [CQ, 1], F32, isOutput=False)
    wk_d = nc.declare_dram_parameter("Wk", [CQ, C], F32, isOutput=False)
    bk_d = nc.declare_dram_parameter("bk", [CQ, 1], F32, isOutput=False)
    wv_d = nc.declare_dram_parameter("Wv", [C, C], F32, isOutput=False)
    bv_d = nc.declare_dram_parameter("bv", [1, C], F32, isOutput=False)
    out_d = nc.declare_dram_parameter("out", [C, T], F32, isOutput=True)

    with tile.TileContext(nc) as tc:
        with (
            tc.tile_pool(name="const", bufs=1) as const,
            tc.tile_pool(name="weights", bufs=1) as wpool,
        ):
            ones_row = const.tile([1, 128], F32)
            nc.gpsimd.memset(ones_row[:], 1.0)
            ones_row_bf = const.tile([1, 128], BF16)
            nc.gpsimd.memset(ones_row_bf[:], 1.0)
            ones_col_bf = const.tile([128, 1], BF16)
            nc.gpsimd.memset(ones_col_bf[:], 1.0)

            # ---- input DMAs.  Sync ring: wq, wk, x0, wv, x1, x2, x3.
            # Bias loads and all transposes go on the ACT ring.
            wq_s = wpool.tile([CQ, C], F32)
            nc.sync.dma_start(out=wq_s[:], in_=wq_d[:])
            wk_s = wpool.tile([CQ, C], F32)
            nc.sync.dma_start(out=wk_s[:], in_=wk_d[:])
            wv_s = wpool.tile([128, NCH, C], F32)
            nc.sync.dma_start(
                out=wv_s[:], in_=wv_d[:].rearrange("(po pi) c -> pi po c", pi=128)
            )
            bqk = wpool.tile([128, 1], F32)    # [bq; bk]
            nc.scalar.dma_start(out=bqk[0:CQ, :], in_=bq_d[:])
            nc.scalar.dma_start(out=bqk[CQ:128, :], in_=bk_d[:])
            bv_row = wpool.tile([1, C], F32)
            nc.scalar.dma_start(out=bv_row[:], in_=bv_d[:])

            x_s = wpool.tile([128, NCH, T], F32)
            x_bf = wpool.tile([128, NCH, T], BF16)
            x_r = x_d[:].rearrange("(po pi) t -> pi po t", pi=128)

            # ---- weights to bf16, transposed through the DMA xbar
            wq_bf = wpool.tile([CQ, C], BF16)
            nc.vector.tensor_copy(wq_bf[:], wq_s[:])
            wk_bf = wpool.tile([CQ, C], BF16)
            nc.vector.tensor_copy(wk_bf[:], wk_s[:])
            wv_bf = wpool.tile([128, NCH, C], BF16)

            nc.vector.tensor_copy(wv_bf[:], wv_s[:])

            wqkT = wpool.tile([128, NCH, 128], BF16)  # [c, chunk, 0:64 WqT | 64:128 WkT]
            wvT = wpool.tile([128, NCH, C], BF16)     # [c, chunk, cout]
            ident_bf = wpool.tile([128, 128], BF16)
            make_identity(nc, ident_bf[:])
            with tc.tile_pool(name="psum_w", bufs=1, space="PSUM") as psum_w:
                for j in range(NCH):
                    ptq = psum_w.tile([128, 2, CQ], BF16, tag="ptq", bufs=2,
                                      name=f"ptq_{j}")
                    nc.tensor.transpose(
                        ptq[:, 0, :], wq_bf[:, j * 128:(j + 1) * 128],
                        ident_bf[0:CQ, 0:CQ]
                    )
                    nc.tensor.transpose(
                        ptq[:, 1, :], wk_bf[:, j * 128:(j + 1) * 128],
                        ident_bf[0:CQ, 0:CQ]
                    )
                    nc.vector.tensor_copy(wqkT[:, j, :], ptq[:])
                for i in range(NCH):       # c chunk of Wv rows
                    ptv = psum_w.tile([128, NCH, 128], BF16, tag="ptv", bufs=2,
                                      name=f"ptv_{i}")
                    for j in range(NCH):   # ch chunk of Wv cols
                        nc.tensor.transpose(
                            ptv[:, j, :], wv_bf[:, i, j * 128:(j + 1) * 128],
                            ident_bf[:]
                        )
                    nc.vector.tensor_copy(
                        wvT[:, :, i * 128:(i + 1) * 128], ptv[:]
                    )

            qk = wpool.tile([128, T], BF16)   # rows 0:64 Q, 64:128 K
            kq0 = wpool.tile([CQ, T], BF16)   # K duplicated onto partitions 0:64
            vT = wpool.tile([128, NSC, C], BF16)
            bv_bcast = wpool.tile([128, C], F32)

            # ---- main pools: 4 score banks + 2+2 AV banks
            with (
                tc.tile_pool(name="et", bufs=2) as et_pool,
                tc.tile_pool(name="ps", bufs=1, space="PSUM") as ps,
                tc.tile_pool(name="small", bufs=2) as small,
                tc.tile_pool(name="ubuf", bufs=2) as ubuf,
                tc.tile_pool(name="outp", bufs=2) as outp,
            ):
                avA = {}
                avB = {}
                ets = {}
                rbs = {}
                stage = {}

                # bv broadcast [1, C] -> [128, C] (borrows an AV bank)
                bvb = ps.tile([128, C], F32, tag="ava0", name="bvb")
                nc.tensor.matmul(
                    bvb[:], ones_row[:], bv_row[:], start=True, stop=True
                )
                nc.vector.tensor_copy(bv_bcast[:], bvb[:])

                def start_block(tb):
                    avA[tb] = [
                        ps.tile([128, TB], F32, tag=f"ava{ck}", name=f"avA{ck}_{tb}")
                        for ck in range(2)
                    ]
                    ets[tb] = et_pool.tile(
                        [128, NSC, TB], BF16, tag="et", name=f"et_{tb}"
                    )

                def emit_scores(tb, jp):
                    tsl = slice(tb * TB, (tb + 1) * TB)
                    j0, j1 = 2 * jp, 2 * jp + 1
                    sc = ps.tile(
                        [128, 2, TB], F32, tag="sc", bufs=2, name=f"sc_{tb}_{jp}"
                    )
                    nc.tensor.matmul(
                        sc[:, 0, :],
                        kq0[:, j0 * 128:(j0 + 1) * 128],
                        qk[0:CQ, tsl],
                        start=True,
                        stop=True,
                    )
                    nc.tensor.matmul(
                        sc[:, 1, :],
                        kq0[:, j1 * 128:(j1 + 1) * 128],
                        qk[0:CQ, tsl],
                        start=True,
                        stop=True,
                    )
                    nc.scalar.activation(
                        ets[tb][:, j0:j0 + 2, :], sc[:, :, :], AF.Exp
                    )

                def emit_consume_A(tb, jp):
                    for idx in (0, 1):
                        j = 2 * jp + idx
                        for ck in range(2):
                            nc.tensor.matmul(
                                avA[tb][ck][:],
                                vT[:, j, ck * 128:(ck + 1) * 128],
                                ets[tb][:, j, :],
                                start=(j == 0),
                                stop=(j == NSC - 1),
                            )

                def emit_consume_B(tb, jp):
                    if jp == 0:
                        avB[tb] = [
                            ps.tile([128, TB], F32, tag=f"avb{ck}",
                                    name=f"avB{ck}_{tb}")
                            for ck in range(2)
                        ]
                    for idx in (0, 1):
                        j = 2 * jp + idx
                        for ck in range(2):
                            nc.tensor.matmul(
                                avB[tb][ck][:],
                                vT[:, j, (2 + ck) * 128:(3 + ck) * 128],
                                ets[tb][:, j, :],
                                start=(j == 0),
                                stop=(j == NSC - 1),
                            )

                def emit_stage(tb, jp):
                    """Staged partial denominator sums for the last block."""
                    et = ets[tb]
                    if jp == 5:
                        s1 = small.tile([128, 4, TB], BF16, tag="s1", bufs=1)
                        nc.vector.tensor_add(s1[:], et[:, 0:4, :], et[:, 4:8, :])
                        s2 = small.tile([128, 2, TB], BF16, tag="s2", bufs=1)
                        nc.vector.tensor_add(s2[:], s1[:, 0:2, :], s1[:, 2:4, :])
                        stage["s2"] = s2
                    elif jp == 6:
                        s3 = small.tile([128, TB], BF16, tag="s3", bufs=1)
                        nc.vector.tensor_add(
                            s3[:], stage["s2"][:, 0, :], stage["s2"][:, 1, :]
                        )
                        s4 = small.tile([128, 2, TB], BF16, tag="s4", bufs=1)
                        nc.vector.tensor_add(s4[:], et[:, 8:10, :], et[:, 10:12, :])
                        s5 = small.tile([128, TB], BF16, tag="s5", bufs=1)
                        nc.vector.tensor_add(s5[:], s4[:, 0, :], s4[:, 1, :])
                        s011 = small.tile([128, TB], BF16, tag="s011", bufs=1)
                        nc.vector.tensor_add(s011[:], s3[:], s5[:])
                        stage["s011"] = s011

                def finish_A(tb):
                    """Drain A-half to SBUF (frees banks fast), then compute
                    the denominator reciprocal and normalize off-path."""
                    tsl = slice(tb * TB, (tb + 1) * TB)
                    uAt = [
                        ubuf.tile([128, TB], F32, tag=f"uA{i}", bufs=1,
                                  name=f"uA{i}_{tb}")
                        for i in range(2)
                    ]
                    for i in range(2):
                        nc.vector.tensor_copy(uAt[i][:], avA[tb][i][:])
                    et = ets[tb]
                    e8 = small.tile([128, 8, TB], BF16, tag="e8", bufs=1,
                                    name=f"e8_{tb}")
                    nc.vector.tensor_add(e8[:], et[:, 0:8, :], et[:, 8:16, :])
                    e4 = small.tile([128, 4, TB], BF16, tag="e4", bufs=1,
                                    name=f"e4_{tb}")
                    nc.vector.tensor_add(e4[:], e8[:, 0:4, :], e8[:, 4:8, :])
                    e2 = small.tile([128, 2, TB], BF16, tag="e2", bufs=1,
                                    name=f"e2_{tb}")
                    nc.vector.tensor_add(e2[:], e4[:, 0:2, :], e4[:, 2:4, :])
                    esum = small.tile([128, TB], F32, tag="esum", bufs=1,
                                      name=f"esum_{tb}")
                    nc.vector.tensor_add(esum[:], e2[:, 0, :], e2[:, 1, :])
                    dsum = small.tile([128, TB], F32, tag="dsum", bufs=1,
                                      name=f"dsum_{tb}")
                    nc.gpsimd.partition_all_reduce(
                        dsum[:], esum[:], channels=128,
                        reduce_op=bass_isa.ReduceOp.add,
                    )
                    rbs[tb] = small.tile([128, TB], F32, tag="rb", name=f"rb_{tb}")
                    nc.vector.reciprocal_approx_fast(rbs[tb][:], dsum[:])
                    for i in range(2):
                        ot = outp.tile(
                            [128, TB], F32, tag=f"ot{i}", name=f"ot{i}_{tb}"
                        )
                        nc.vector.tensor_mul(ot[:], uAt[i][:], rbs[tb][:])
                        nc.sync.dma_start(
                            out=out_d[i * 128:(i + 1) * 128, tsl], in_=ot[:]
                        )

                def finish_B(tb):
                    tsl = slice(tb * TB, (tb + 1) * TB)
                    uBt = [
                        ubuf.tile([128, TB], F32, tag=f"uB{i}", bufs=1,
                                  name=f"uB{i}_{tb}")
                        for i in range(2)
                    ]
                    for i in range(2):
                        nc.vector.tensor_copy(uBt[i][:], avB[tb][i][:])
                    for i in range(2):
                        ck = 2 + i
                        ot = outp.tile(
                            [128, TB], F32, tag=f"ot{ck}", name=f"ot{ck}_{tb}"
                        )
                        nc.vector.tensor_mul(ot[:], uBt[i][:], rbs[tb][:])
                        nc.sync.dma_start(
                            out=out_d[ck * 128:(ck + 1) * 128, tsl], in_=ot[:]
                        )

                def tail_last(tb):
                    """Tail for the final block: B-pairs interleave with the
                    PE denominator fold; muls read PSUM directly."""
                    tsl = slice(tb * TB, (tb + 1) * TB)
                    et = ets[tb]
                    # DVE: finish the staged denominator sum (waits last exp)
                    p1 = small.tile([128, 2, TB], BF16, tag="p1", bufs=1)
                    nc.vector.tensor_add(p1[:], et[:, 12:14, :], et[:, 14:16, :])
                    p2 = small.tile([128, TB], BF16, tag="p2", bufs=1)
                    nc.vector.tensor_add(p2[:], p1[:, 0, :], p1[:, 1, :])
                    esum = small.tile([128, TB], BF16, tag="esumL", bufs=1)
                    nc.vector.tensor_add(esum[:], stage["s011"][:], p2[:])
                    # PE: remaining B pairs around the denominator fold
                    emit_consume_B(tb, 4)
                    emit_consume_B(tb, 5)
                    dnrb = ps.tile([128, 2, TB], F32, tag="sc", bufs=2, name="dnrb")
                    nc.tensor.matmul(
                        dnrb[0:1, 0, :], ones_col_bf[:], esum[:],
                        start=True, stop=True,
                    )
                    dnrow = small.tile([1, TB], F32, tag="dnrow", bufs=1)
                    nc.scalar.copy(dnrow[:], dnrb[0:1, 0, :])
                    rcol = small.tile([1, TB], F32, tag="rcol", bufs=1)
                    nc.vector.reciprocal_approx_fast(rcol[:], dnrow[:])
                    rcol_bf = small.tile([1, TB], BF16, tag="rcolbf", bufs=1)
                    nc.vector.tensor_copy(rcol_bf[:], rcol[:])
                    emit_consume_B(tb, 6)
                    emit_consume_B(tb, 7)
                    nc.tensor.matmul(
                        dnrb[:, 1, :], ones_row_bf[:], rcol_bf[:],
                        start=True, stop=True,
                    )
                    rb = small.tile([128, TB], F32, tag="rbL", bufs=1)
                    nc.scalar.copy(rb[:], dnrb[:, 1, :])
                    for ck in range(4):
                        av = avA[tb][ck] if ck < 2 else avB[tb][ck - 2]
                        ot = outp.tile(
                            [128, TB], F32, tag=f"ot{ck}", name=f"otL{ck}_{tb}"
                        )
                        nc.vector.tensor_mul(ot[:], av[:], rb[:])
                        nc.sync.dma_start(
                            out=out_d[ck * 128:(ck + 1) * 128, tsl], in_=ot[:]
                        )

                # ---- preamble: x chunks with projections + t-block 0 chasing
                pending = None
                start_block(0)
                for m in range(NTB):
                    msl = slice(m * TB, (m + 1) * TB)
                    nc.sync.dma_start(out=x_s[:, :, msl], in_=x_r[:, :, msl])
                    nc.vector.tensor_copy(
                        x_bf[:, :, m * TB:m * TB + 256], x_s[:, :, m * TB:m * TB + 256]
                    )
                    nc.scalar.activation(
                        x_bf[:, :, m * TB + 256:(m + 1) * TB],
                        x_s[:, :, m * TB + 256:(m + 1) * TB],
                        AF.Copy,
                    )

                    # packed Q/K projection; ACT writes qk and the kq0 dup
                    qkp = ps.tile([128, 2, TB], F32, tag="sc", bufs=2,
                                  name=f"qkp_{m}")
                    for ci in range(NCH):
                        nc.tensor.matmul(
                            qkp[:, 0, :], wqkT[:, ci, :], x_bf[:, ci, msl],
                            start=(ci == 0), stop=(ci == NCH - 1),
                        )
                    nc.scalar.add(qk[:, msl], qkp[:, 0, :], bqk[:, 0:1])
                    nc.scalar.add(kq0[:, msl], qkp[CQ:128, 0, :], bqk[CQ:128, 0:1])

                    # t-block 0 scores chase the projections
                    for jp in (2 * m, 2 * m + 1):
                        emit_scores(0, jp)

                    # V^T projection for this chunk's four s-chunks
                    for j in range(4 * m, 4 * m + 4):
                        psv = ps.tile([128, C], F32, tag=f"avb{j % 2}",
                                      name=f"vp_{j}")
                        for ci in range(NCH):
                            nc.tensor.matmul(
                                psv[:],
                                x_bf[:, ci, j * 128:(j + 1) * 128],
                                wvT[:, ci, :],
                                start=(ci == 0),
                                stop=(ci == NCH - 1),
                            )
                        nc.vector.tensor_add(vT[:, j, :], psv[:], bv_bcast[:])

                    # consume t-block 0 pairs one step behind
                    for jp in (2 * m, 2 * m + 1):
                        if pending is not None:
                            emit_consume_A(*pending)
                        pending = (0, jp)

                # ---- main phases
                for tb in range(1, NTB):
                    start_block(tb)
                    for jp in range(NPAIR):
                        emit_scores(tb, jp)
                        if tb == NTB - 1:
                            emit_stage(tb, jp)
                        ptb, pjp = pending
                        emit_consume_A(ptb, pjp)
                        if pjp == NPAIR - 1:
                            finish_A(ptb)
                        pending = (tb, jp)
                        if tb < NTB - 1:
                            emit_consume_B(tb - 1, jp)
                            if jp == NPAIR - 1:
                                finish_B(tb - 1)
                        else:
                            # last phase: catch up — B(2) at 2 pairs/step,
                            # then start B(3)
                            if jp < 4:
                                emit_consume_B(tb - 1, 2 * jp)
                                emit_consume_B(tb - 1, 2 * jp + 1)
                                if jp == 3:
                                    finish_B(tb - 1)
                            else:
                                emit_consume_B(tb, jp - 4)
                # tail
                ptb, pjp = pending
                emit_consume_A(ptb, pjp)
                tail_last(NTB - 1)

    nc.compile()
    return nc


_PROGRAM = None


def _get_program() -> bass.Bass:
    global _PROGRAM
    if _PROGRAM is None:
        _PROGRAM = _build_program()
    return _PROGRAM


def kernel(**inputs: np.ndarray) -> np.ndarray:
    x = np.ascontiguousarray(np.asarray(inputs["x"], dtype=np.float32))
    wq = np.ascontiguousarray(np.asarray(inputs["Wq"], dtype=np.float32))
    bq = np.ascontiguousarray(np.asarray(inputs["bq"], dtype=np.float32)).reshape(CQ, 1)
    wk = np.ascontiguousarray(np.asarray(inputs["Wk"], dtype=np.float32))
    bk = np.ascontiguousarray(np.asarray(inputs["bk"], dtype=np.float32)).reshape(CQ, 1)
    wv = np.ascontiguousarray(np.asarray(inputs["Wv"], dtype=np.float32))
    bv = np.ascontiguousarray(np.asarray(inputs["bv"], dtype=np.float32)).reshape(1, C)

    nc = _get_program()
    in_maps = [
        {
            "x": np.ascontiguousarray(x[b]),
            "Wq": wq,
            "bq": bq,
            "Wk": wk,
            "bk": bk,
            "Wv": wv,
            "bv": bv,
        }
        for b in range(NCORES)
    ]
    res = run_bass_kernel_spmd(nc, in_maps, list(range(NCORES)))
    out = np.stack([res.results[b]["out"] for b in range(NCORES)], axis=0)
    return out.astype(np.float32)


if __name__ == "__main__":
    import reference

    inputs = {k: np.asarray(v) for k, v in reference.setup_inputs().items()}
    expected = np.asarray(reference.reference(**inputs))
    actual = kernel(**inputs)
    rel = np.linalg.norm(actual - expected) / np.linalg.norm(expected)
    print("Relative error:", rel)


# revision 15
# speedup vs baseline: 1.0990x; 1.0027x over previous
"""Trainium2 Bass kernel for per-batch channel attention (CxAM-style).

Reference (per batch element b):
    q = (Wq @ x_b + bq)        # [64, T]
    k = (Wk @ x_b + bk)        # [64, T]
    v = (Wv @ x_b + bv)        # [512, T]
    R = q.T @ k                # [T, T]
    A = softmax(R, axis=-1)
    out_b = v @ A.T            # [512, T]

Sharding: pure data-parallel — batch B=8, one batch element per NeuronCore.

Per-core algorithm:
    x is DMA'd in four t-chunks (all channels per chunk) so the Q/K/V
    projections and t-block 0's score/AV pipeline chase the DMA.  All
    weight transposes go through the DMA xbar (dma_start_transpose, bf16)
    so the PE never transposes anything.  The K rows of the packed Q/K
    projection are written twice by ACT (once into qk rows 64:128, once
    onto partitions 0:64 as kq0) so score matmuls have both operands on
    matching partitions without any SBUF-duplication DMA.
    Per t-block of 512, per s-chunk pair:
      ST_j [s=128, t=512] = K_chunk.T @ Q_block      (scores, transposed)
      E_j = exp(ST_j)   one batched ACT op per pair  (bf16)
      U_ck [c=128, t] += VT_chunk_ck.T @ E_j         (unnormalized out)
    The U accumulation is split into halves A=(ck0,ck1)/B=(ck2,ck3); B of
    block tb runs during block tb+1, so only 2+2 PSUM banks are needed
    (4 remain for double-buffered score pairs).  When a half finishes it
    is immediately copied PSUM->SBUF, freeing the banks; normalization
    happens off the critical path: a DVE pairwise tree sums the 16 E_j
    tiles, a GPSIMD partition_all_reduce folds the 128 partitions
    (broadcasting the result), a DVE fast reciprocal gives rb, and
    out = U_sbuf * rb.  The last block instead uses staged partial tree
    sums plus a PE ones-matmul fold/broadcast (PSUM is free by then) so
    the tail chain after the final exp is short.  Output DMAs ride the
    sync ring (idle after the input phase) so they never delay the exps.
"""

import os

os.environ.setdefault("MYCRO_LOCAL_CACHE", "1")

import numpy as np

import concourse.bass as bass
import concourse.mybir as mybir
import concourse.tile as tile
from concourse import bacc
from concourse import bass_isa
from concourse.bass_utils import run_bass_kernel_spmd
from concourse.masks import make_identity

F32 = mybir.dt.float32
BF16 = mybir.dt.bfloat16
AF = mybir.ActivationFunctionType

B = 8
C = 512
T = 2048
CQ = 64
NCORES = 8

TB = 512            # t-block (free dim of main matmuls)
NTB = T // TB       # 4
NSC = T // 128      # 16 s-chunks
NPAIR = NSC // 2    # 8 score pairs per t-block
NCH = C // 128      # 4 contraction chunks


def _build_program() -> bass.Bass:
    nc = bacc.Bacc("TRN2", target_bir_lowering=False, debug=False, num_devices=NCORES)

    x_d = nc.declare_dram_parameter("x", [C, T], F32, isOutput=False)
    wq_d = nc.declare_dram_parameter("Wq", [CQ, C], F32, isOutput=False)
    bq_d = nc.declare_dram_parameter("bq", [CQ, 1], F32, isOutput=False)
    wk_d = nc.declare_dram_parameter("Wk", [CQ, C], F32, isOutput=False)
    bk_d = nc.declare_dram_parameter("bk", [CQ, 1], F32, isOutput=False)
    wv_d = nc.declare_dram_parameter("Wv", [C, C], F32, isOutput=False)
    bv_d = nc.declare_dram_parameter("bv", [1, C], F32, isOutput=False)
    out_d = nc.declare_dram_parameter("out", [C, T], F32, isOutput=True)

    with tile.TileContext(nc) as tc:
        with (
            tc.tile_pool(name="const", bufs=1) as const,
            tc.tile_pool(name="weights", bufs=1) as wpool,
        ):
            ones_row = const.tile([1, 128], F32)
            nc.gpsimd.memset(ones_row[:], 1.0)
            ones_row_bf = const.tile([1, 128], BF16)
            nc.gpsimd.memset(ones_row_bf[:], 1.0)
            ones_col_bf = const.tile([128, 1], BF16)
            nc.gpsimd.memset(ones_col_bf[:], 1.0)

            # ---- input DMAs.  Sync ring: wq, wk, x0, wv, x1, x2, x3.
            # Bias loads and all transposes go on the ACT ring.
            wq_s = wpool.tile([CQ, C], F32)
            nc.sync.dma_start(out=wq_s[:], in_=wq_d[:])
            wk_s = wpool.tile([CQ, C], F32)
            nc.sync.dma_start(out=wk_s[:], in_=wk_d[:])
            wv_s = wpool.tile([128, NCH, C], F32)
            nc.sync.dma_start(
                out=wv_s[:], in_=wv_d[:].rearrange("(po pi) c -> pi po c", pi=128)
            )
            bqk = wpool.tile([128, 1], F32)    # [bq; bk]
            nc.scalar.dma_start(out=bqk[0:CQ, :], in_=bq_d[:])
            nc.scalar.dma_start(out=bqk[CQ:128, :], in_=bk_d[:])
            bv_row = wpool.tile([1, C], F32)
            nc.scalar.dma_start(out=bv_row[:], in_=bv_d[:])

            x_s = wpool.tile([128, NCH, T], F32)
            x_bf = wpool.tile([128, NCH, T], BF16)
            x_r = x_d[:].rearrange("(po pi) t -> pi po t", pi=128)

            # ---- weights to bf16, transposed through the DMA xbar
            wq_bf = wpool.tile([CQ, C], BF16)
            nc.vector.tensor_copy(wq_bf[:], wq_s[:])
            wk_bf = wpool.tile([CQ, C], BF16)
            nc.vector.tensor_copy(wk_bf[:], wk_s[:])
            wv_bf = wpool.tile([128, NCH, C], BF16)

            wqkT = wpool.tile([128, NCH, 128], BF16)  # [c, chunk, 0:64 WqT | 64:128 WkT]
            wvT = wpool.tile([128, NCH, C], BF16)     # [c, chunk, cout]
            ident_bf = wpool.tile([128, 128], BF16)
            make_identity(nc, ident_bf[:])
            with tc.tile_pool(name="psum_w", bufs=1, space="PSUM") as psum_w:
                for j in range(NCH):
                    ptq = psum_w.tile([128, 2, CQ], BF16, tag="ptq", bufs=2,
                                      name=f"ptq_{j}")
                    nc.tensor.transpose(
                        ptq[:, 0, :], wq_bf[:, j * 128:(j + 1) * 128],
                        ident_bf[0:CQ, 0:CQ]
                    )
                    nc.tensor.transpose(
                        ptq[:, 1, :], wk_bf[:, j * 128:(j + 1) * 128],
                        ident_bf[0:CQ, 0:CQ]
                    )
                    nc.vector.tensor_copy(wqkT[:, j, :], ptq[:])
                nc.vector.tensor_copy(wv_bf[:], wv_s[:])
                for i in range(NCH):       # c chunk of Wv rows
                    ptv = psum_w.tile([128, NCH, 128], BF16, tag="ptv", bufs=2,
                                      name=f"ptv_{i}")
                    for j in range(NCH):   # ch chunk of Wv cols
                        nc.tensor.transpose(
                            ptv[:, j, :], wv_bf[:, i, j * 128:(j + 1) * 128],
                            ident_bf[:]
                        )
                    nc.vector.tensor_copy(
                        wvT[:, :, i * 128:(i + 1) * 128], ptv[:]
                    )

            qk = wpool.tile([128, T], BF16)   # rows 0:64 Q, 64:128 K
            kq0 = wpool.tile([CQ, T], BF16)   # K duplicated onto partitions 0:64
            vT = wpool.tile([128, NSC, C], BF16)
            bv_bcast = wpool.tile([128, C], F32)

            # ---- main pools: 4 score banks + 2+2 AV banks
            with (
                tc.tile_pool(name="et", bufs=2) as et_pool,
                tc.tile_pool(name="ps", bufs=1, space="PSUM") as ps,
                tc.tile_pool(name="small", bufs=2) as small,
                tc.tile_pool(name="ubuf", bufs=2) as ubuf,
                tc.tile_pool(name="outp", bufs=2) as outp,
            ):
                avA = {}
                avB = {}
                ets = {}
                rbs = {}
                stage = {}

                # bv broadcast [1, C] -> [128, C] (borrows an AV bank)
                bvb = ps.tile([128, C], F32, tag="ava0", name="bvb")
                nc.tensor.matmul(
                    bvb[:], ones_row[:], bv_row[:], start=True, stop=True
                )
                nc.vector.tensor_copy(bv_bcast[:], bvb[:])

                def start_block(tb):
                    avA[tb] = [
                        ps.tile([128, TB], F32, tag=f"ava{ck}", name=f"avA{ck}_{tb}")
                        for ck in range(2)
                    ]
                    ets[tb] = et_pool.tile(
                        [128, NSC, TB], BF16, tag="et", name=f"et_{tb}"
                    )

                def emit_scores(tb, jp):
                    tsl = slice(tb * TB, (tb + 1) * TB)
                    j0, j1 = 2 * jp, 2 * jp + 1
                    sc = ps.tile(
                        [128, 2, TB], F32, tag="sc", bufs=2, name=f"sc_{tb}_{jp}"
                    )
                    nc.tensor.matmul(
                        sc[:, 0, :],
                        kq0[:, j0 * 128:(j0 + 1) * 128],
                        qk[0:CQ, tsl],
                        start=True,
                        stop=True,
                    )
                    nc.tensor.matmul(
                        sc[:, 1, :],
                        kq0[:, j1 * 128:(j1 + 1) * 128],
                        qk[0:CQ, tsl],
                        start=True,
                        stop=True,
                    )
                    nc.scalar.activation(
                        ets[tb][:, j0:j0 + 2, :], sc[:, :, :], AF.Exp
                    )

                def emit_consume_A(tb, jp):
                    for idx in (0, 1):
                        j = 2 * jp + idx
                        for ck in range(2):
                            nc.tensor.matmul(
                                avA[tb][ck][:],
                                vT[:, j, ck * 128:(ck + 1) * 128],
                                ets[tb][:, j, :],
                                start=(j == 0),
                                stop=(j == NSC - 1),
                            )

                def emit_consume_B(tb, jp):
                    if jp == 0:
                        avB[tb] = [
                            ps.tile([128, TB], F32, tag=f"avb{ck}",
                                    name=f"avB{ck}_{tb}")
                            for ck in range(2)
                        ]
                    for idx in (0, 1):
                        j = 2 * jp + idx
                        for ck in range(2):
                            nc.tensor.matmul(
                                avB[tb][ck][:],
                                vT[:, j, (2 + ck) * 128:(3 + ck) * 128],
                                ets[tb][:, j, :],
                                start=(j == 0),
                                stop=(j == NSC - 1),
                            )

                def emit_stage(tb, jp):
                    """Staged partial denominator sums for the last block."""
                    et = ets[tb]
                    if jp == 5:
                        s1 = small.tile([128, 4, TB], BF16, tag="s1", bufs=1)
                        nc.vector.tensor_add(s1[:], et[:, 0:4, :], et[:, 4:8, :])
                        s2 = small.tile([128, 2, TB], BF16, tag="s2", bufs=1)
                        nc.vector.tensor_add(s2[:], s1[:, 0:2, :], s1[:, 2:4, :])
                        stage["s2"] = s2
                    elif jp == 6:
                        s3 = small.tile([128, TB], BF16, tag="s3", bufs=1)
                        nc.vector.tensor_add(
                            s3[:], stage["s2"][:, 0, :], stage["s2"][:, 1, :]
                        )
                        s4 = small.tile([128, 2, TB], BF16, tag="s4", bufs=1)
                        nc.vector.tensor_add(s4[:], et[:, 8:10, :], et[:, 10:12, :])
                        s5 = small.tile([128, TB], BF16, tag="s5", bufs=1)
                        nc.vector.tensor_add(s5[:], s4[:, 0, :], s4[:, 1, :])
                        s011 = small.tile([128, TB], BF16, tag="s011", bufs=1)
                        nc.vector.tensor_add(s011[:], s3[:], s5[:])
                        stage["s011"] = s011

                def finish_A(tb):
                    """Drain A-half to SBUF (frees banks fast), then compute
                    the denominator reciprocal and normalize off-path."""
                    tsl = slice(tb * TB, (tb + 1) * TB)
                    uAt = [
                        ubuf.tile([128, TB], F32, tag=f"uA{i}", bufs=1,
                                  name=f"uA{i}_{tb}")
                        for i in range(2)
                    ]
                    for i in range(2):
                        nc.vector.tensor_copy(uAt[i][:], avA[tb][i][:])
                    et = ets[tb]
                    e8 = small.tile([128, 8, TB], BF16, tag="e8", bufs=1,
                                    name=f"e8_{tb}")
                    nc.vector.tensor_add(e8[:], et[:, 0:8, :], et[:, 8:16, :])
                    e4 = small.tile([128, 4, TB], BF16, tag="e4", bufs=1,
                                    name=f"e4_{tb}")
                    nc.vector.tensor_add(e4[:], e8[:, 0:4, :], e8[:, 4:8, :])
                    e2 = small.tile([128, 2, TB], BF16, tag="e2", bufs=1,
                                    name=f"e2_{tb}")
                    nc.vector.tensor_add(e2[:], e4[:, 0:2, :], e4[:, 2:4, :])
                    esum = small.tile([128, TB], F32, tag="esum", bufs=1,
                                      name=f"esum_{tb}")
                    nc.vector.tensor_add(esum[:], e2[:, 0, :], e2[:, 1, :])
                    dsum = small.tile([128, TB], F32, tag="dsum", bufs=1,
                                      name=f"dsum_{tb}")
                    nc.gpsimd.partition_all_reduce(
                        dsum[:], esum[:], channels=128,
                        reduce_op=bass_isa.ReduceOp.add,
                    )
                    rbs[tb] = small.tile([128, TB], F32, tag="rb", name=f"rb_{tb}")
                    nc.vector.reciprocal_approx_fast(rbs[tb][:], dsum[:])
                    for i in range(2):
                        ot = outp.tile(
                            [128, TB], F32, tag=f"ot{i}", name=f"ot{i}_{tb}"
                        )
                        nc.vector.tensor_mul(ot[:], uAt[i][:], rbs[tb][:])
                        nc.sync.dma_start(
                            out=out_d[i * 128:(i + 1) * 128, tsl], in_=ot[:]
                        )

                def finish_B(tb):
                    tsl = slice(tb * TB, (tb + 1) * TB)
                    uBt = [
                        ubuf.tile([128, TB], F32, tag=f"uB{i}", bufs=1,
                                  name=f"uB{i}_{tb}")
                        for i in range(2)
                    ]
                    for i in range(2):
                        nc.vector.tensor_copy(uBt[i][:], avB[tb][i][:])
                    for i in range(2):
                        ck = 2 + i
                        ot = outp.tile(
                            [128, TB], F32, tag=f"ot{ck}", name=f"ot{ck}_{tb}"
                        )
                        nc.vector.tensor_mul(ot[:], uBt[i][:], rbs[tb][:])
                        nc.sync.dma_start(
                            out=out_d[ck * 128:(ck + 1) * 128, tsl], in_=ot[:]
                        )

                def tail_last(tb):
                    """Tail for the final block: B-pairs interleave with the
                    PE denominator fold; U drains to bf16 SBUF while the
                    reciprocal chain runs, then 2x-mode muls."""
                    tsl = slice(tb * TB, (tb + 1) * TB)
                    et = ets[tb]
                    # DVE: finish the staged denominator sum (waits last exp)
                    p1 = small.tile([128, 2, TB], BF16, tag="p1", bufs=1)
                    nc.vector.tensor_add(p1[:], et[:, 12:14, :], et[:, 14:16, :])
                    p2 = small.tile([128, TB], BF16, tag="p2", bufs=1)
                    nc.vector.tensor_add(p2[:], p1[:, 0, :], p1[:, 1, :])
                    esum = small.tile([128, TB], BF16, tag="esumL", bufs=1)
                    nc.vector.tensor_add(esum[:], stage["s011"][:], p2[:])
                    uL = [
                        ubuf.tile([128, TB], BF16, tag=f"uL{ck}", bufs=1,
                                  name=f"uL{ck}_{tb}")
                        for ck in range(4)
                    ]
                    nc.vector.tensor_copy(uL[0][:], avA[tb][0][:])
                    nc.vector.tensor_copy(uL[1][:], avA[tb][1][:])
                    # PE: remaining B pairs around the denominator fold
                    emit_consume_B(tb, 4)
                    emit_consume_B(tb, 5)
                    dnrb = ps.tile([128, 2, TB], F32, tag="sc", bufs=2, name="dnrb")
                    nc.tensor.matmul(
                        dnrb[0:1, 0, :], ones_col_bf[:], esum[:],
                        start=True, stop=True,
                    )
                    dnrow = small.tile([1, TB], F32, tag="dnrow", bufs=1)
                    nc.scalar.copy(dnrow[:], dnrb[0:1, 0, :])
                    rcol = small.tile([1, TB], F32, tag="rcol", bufs=1)
                    nc.vector.reciprocal_approx_fast(rcol[:], dnrow[:])
                    rcol_bf = small.tile([1, TB], BF16, tag="rcolbf", bufs=1)
                    nc.vector.tensor_copy(rcol_bf[:], rcol[:])
                    emit_consume_B(tb, 6)
                    emit_consume_B(tb, 7)
                    nc.tensor.matmul(
                        dnrb[:, 1, :], ones_row_bf[:], rcol_bf[:],
                        start=True, stop=True,
                    )
                    nc.vector.tensor_copy(uL[2][:], avB[tb][0][:])
                    nc.vector.tensor_copy(uL[3][:], avB[tb][1][:])
                    rb = small.tile([128, TB], BF16, tag="rbL", bufs=1)
                    nc.vector.tensor_copy(rb[:], dnrb[:, 1, :])
                    for ck in range(4):
                        ot = outp.tile(
                            [128, TB], F32, tag=f"ot{ck}", name=f"otL{ck}_{tb}"
                        )
                        nc.vector.tensor_mul(ot[:], uL[ck][:], rb[:])
                        ring = nc.sync if ck % 2 == 0 else nc.scalar
                        ring.dma_start(
                            out=out_d[ck * 128:(ck + 1) * 128, tsl], in_=ot[:]
                        )

                # ---- preamble: x chunks with projections + t-block 0 chasing
                pending = None
                start_block(0)
                for m in range(NTB):
                    msl = slice(m * TB, (m + 1) * TB)
                    nc.sync.dma_start(out=x_s[:, :, msl], in_=x_r[:, :, msl])
                    nc.vector.tensor_copy(
                        x_bf[:, :, m * TB:m * TB + 256], x_s[:, :, m * TB:m * TB + 256]
                    )
                    nc.scalar.activation(
                        x_bf[:, :, m * TB + 256:(m + 1) * TB],
                        x_s[:, :, m * TB + 256:(m + 1) * TB],
                        AF.Copy,
                    )

                    # packed Q/K projection; ACT writes qk and the kq0 dup
                    qkp = ps.tile([128, 2, TB], F32, tag="sc", bufs=2,
                                  name=f"qkp_{m}")
                    for ci in range(NCH):
                        nc.tensor.matmul(
                            qkp[:, 0, :], wqkT[:, ci, :], x_bf[:, ci, msl],
                            start=(ci == 0), stop=(ci == NCH - 1),
                        )
                    nc.scalar.add(qk[:, msl], qkp[:, 0, :], bqk[:, 0:1])
                    nc.scalar.add(kq0[:, msl], qkp[CQ:128, 0, :], bqk[CQ:128, 0:1])

                    # t-block 0 scores chase the projections
                    for jp in (2 * m, 2 * m + 1):
                        emit_scores(0, jp)

                    # V^T projection for this chunk's four s-chunks
                    for j in range(4 * m, 4 * m + 4):
                        psv = ps.tile([128, C], F32, tag=f"avb{j % 2}",
                                      name=f"vp_{j}")
                        for ci in range(NCH):
                            nc.tensor.matmul(
                                psv[:],
                                x_bf[:, ci, j * 128:(j + 1) * 128],
                                wvT[:, ci, :],
                                start=(ci == 0),
                                stop=(ci == NCH - 1),
                            )
                        nc.vector.tensor_add(vT[:, j, :], psv[:], bv_bcast[:])

                    # consume t-block 0 pairs one step behind
                    for jp in (2 * m, 2 * m + 1):
                        if pending is not None:
                            emit_consume_A(*pending)
                        pending = (0, jp)

                # ---- main phases
                for tb in range(1, NTB):
                    start_block(tb)
                    for jp in range(NPAIR):
                        emit_scores(tb, jp)
                        if tb == NTB - 1:
                            emit_stage(tb, jp)
                        ptb, pjp = pending
                        emit_consume_A(ptb, pjp)
                        if pjp == NPAIR - 1:
                            finish_A(ptb)
                        pending = (tb, jp)
                        if tb < NTB - 1:
                            emit_consume_B(tb - 1, jp)
                            if jp == NPAIR - 1:
                                finish_B(tb - 1)
                        else:
                            # last phase: catch up — B(2) at 2 pairs/step,
                            # then start B(3)
                            if jp < 4:
                                emit_consume_B(tb - 1, 2 * jp)
                                emit_consume_B(tb - 1, 2 * jp + 1)
                                if jp == 3:
                                    finish_B(tb - 1)
                            else:
                                emit_consume_B(tb, jp - 4)
                # tail
                ptb, pjp = pending
                emit_consume_A(ptb, pjp)
                tail_last(NTB - 1)

    nc.compile()
    return nc


_PROGRAM = None


def _get_program() -> bass.Bass:
    global _PROGRAM
    if _PROGRAM is None:
        _PROGRAM = _build_program()
    return _PROGRAM


def kernel(**inputs: np.ndarray) -> np.ndarray:
    x = np.ascontiguousarray(np.asarray(inputs["x"], dtype=np.float32))
    wq = np.ascontiguousarray(np.asarray(inputs["Wq"], dtype=np.float32))
    bq = np.ascontiguousarray(np.asarray(inputs["bq"], dtype=np.float32)).reshape(CQ, 1)
    wk = np.ascontiguousarray(np.asarray(inputs["Wk"], dtype=np.float32))
    bk = np.ascontiguousarray(np.asarray(inputs["bk"], dtype=np.float32)).reshape(CQ, 1)
    wv = np.ascontiguousarray(np.asarray(inputs["Wv"], dtype=np.float32))
    bv = np.ascontiguousarray(np.asarray(inputs["bv"], dtype=np.float32)).reshape(1, C)

    nc = _get_program()
    in_maps = [
        {
            "x": np.ascontiguousarray(x[b]),
            "Wq": wq,
            "bq": bq,
            "Wk": wk,
            "bk": bk,
            "Wv": wv,
            "bv": bv,
        }
        for b in range(NCORES)
    ]
    res = run_bass_kernel_spmd(nc, in_maps, list(range(NCORES)))
    out = np.stack([res.results[b]["out"] for b in range(NCORES)], axis=0)
    return out.astype(np.float32)


if __name__ == "__main__":
    import reference

    inputs = {k: np.asarray(v) for k, v in reference.setup_inputs().items()}
    expected = np.asarray(reference.reference(**inputs))
    actual = kernel(**inputs)
    rel = np.linalg.norm(actual - expected) / np.linalg.norm(expected)
    print("Relative error:", rel)
